# revision 28
# baseline (speedup 1.0000x reference)
"""AttentionBlock (GroupNorm + single-head spatial attention + proj + residual)
on 8 trn2 NeuronCores, data-parallel over the batch (1 image per core).

Final design (~55us vs ~94us baseline), measured on HW via test.py:
  - proj_w folded into W_v host-side (attention output is linear in v), so
    the proj GEMM stage disappears; q/k folded (t = M^T xn, M = Wq^T Wk).
  - All four big GEMMs (t, S, v~, att) in fp8 e4m3 DoubleRow perf mode:
    K=256 per instruction at 0.5 PE cycles/row -> 216ns per
    [256]x128x512 matmul at full clock (2x the f32r MAC rate).  PSUM
    accumulation stays fp32.  Measured end-to-end rel err 7.8e-3 against
    the fp32 reference (tolerance 2e-2).
  - Attention output computed transposed (att^T[i,c], pixels on
    partitions): the softmax denominators land in [128,1] psums via
    1-wide matmuls against a ones vector, the reciprocal is a [128,1]
    vector op, and the final evacuation is one scalar_tensor_tensor per
    pixel tile: (att_psum * 1/D) + (x^T + pb), with the residual + proj
    bias pre-added host-side and DMA'd (bf16) off the critical path.
  - x streams in fp8 (used only for GroupNorm stats + xn; the residual
    uses the separately-DMA'd x^T) in 4 chunks -- per-chunk ring overhead
    ~0.6us dominates smaller chunks.  y streams out in bf16.
  - GroupNorm is pipelined per 128-channel tile (groups of 16 channels
    never span tiles): bn_stats as each chunk lands (vector), group
    combine via tiny indicator matmuls (PE), the rstd chain spread over
    scalar/gpsimd/vector so no in-order engine queue serializes it, and
    the heavy fp8 xn writes placed where they cannot sit ahead of a later
    tile's chain ops (gpsimd inline for ct0/1, scalar after the loop).
  - The scalar engine's activation table holds one function family; a
    reload costs 1.28us.  A dummy Sqrt at kernel start pre-triggers the
    Sqrt table during the warm-up window, and keeping the t/v~ psum
    evacuations on vector means the Exp reload fires in a scalar-idle
    window instead of on the critical path.
  - The PE clock gate (HAM) halves the clock after ~1us of PE idle and
    needs ~3-5us of sustained work to reopen.  Cheap 64-wide bf16 filler
    matmuls (~53ns each), WAW-chained and batch-anchored into the
    GroupNorm phase, keep it open through the DMA/stats window.  The
    engines execute strictly in order at runtime; placement is decided by
    the compile-time list scheduler, so filler position is controlled by
    issue point + dependency anchors, and sizes are tuned so a batch
    never overruns into a ready real matmul.
"""

import sys

sys.path.insert(0, "/opt/trn_rl_repo")

import numpy as np

import concourse.bass as bass
import concourse.tile as tile
from concourse import bacc, mybir
from concourse.bass_utils import run_bass_kernel_spmd
from concourse.tile_rust import add_dep_helper

F32 = mybir.dt.float32
BF16 = mybir.dt.bfloat16
FP8 = mybir.dt.float8e4  # e4m3
DR = mybir.MatmulPerfMode.DoubleRow

C = 512          # channels
NPIX = 1024      # pixels per image (32*32)
CT = 4           # channel tiles of 128
JT = 8           # pixel tiles of 128
NH = 2           # halves of NPIX for the 512-wide moving dim
G = 32           # groups
GS = 16          # channels per group
GPT = 8          # groups per channel tile (128/16)
EPS = 1e-5
SCALE = C ** -0.5
WARM0 = 60       # idle-filler matmuls before the first gn matmul
WARMI = 20       # idle-filler matmuls per channel tile
WARMT = 45       # idle-filler matmuls before the t stage

TRACE = False          # set True (from test.py) to capture an NTFF profile
TRACE_KW = {}          # extra kwargs for run_bass_kernel_spmd
LAST_RESULTS = None    # BassKernelResults of the most recent run

_cache = {}


def _build(with_qbias=False):
    nc = bacc.Bacc("TRN2")

    x_d = nc.dram_tensor("x", [128, CT, NPIX], FP8, kind="ExternalInput")
    xpb_d = nc.dram_tensor("xpb", [128, JT, C], BF16, kind="ExternalInput")
    qa_d = nc.dram_tensor("qa", [128, CT, C], FP8, kind="ExternalInput")
    vw_d = nc.dram_tensor("vw", [128, CT, C], FP8, kind="ExternalInput")
    gnw_d = nc.dram_tensor("gnw", [128, CT], F32, kind="ExternalInput")
    gnb_d = nc.dram_tensor("gnb", [128, CT], F32, kind="ExternalInput")
    if with_qbias:
        rw_d = nc.dram_tensor("rw", [128, CT, 1], FP8, kind="ExternalInput")
    y_d = nc.dram_tensor("y", [128, JT, C], BF16, kind="ExternalOutput")

    # Group indicators: within every 128-channel tile the 8 groups are the
    # consecutive 16-channel blocks, identically for each tile.
    ind1 = np.zeros((128, GPT), np.float32)   # group reduce (pre-scaled 1/GS)
    for p in range(128):
        ind1[p, p // GS] = 1.0 / GS
    ind2 = np.zeros((GPT, 128), np.float32)   # broadcast back to channels
    for p in range(128):
        ind2[p // GS, p] = 1.0
    ind1_d = nc.inline_tensor(ind1, name="ind1")
    ind2_d = nc.inline_tensor(ind2, name="ind2")

    with tile.TileContext(nc) as tc:
        with (
            nc.allow_low_precision(reason="fp8 attention path, tol 2e-2"),
            tc.tile_pool(name="persist", bufs=1) as pers,
            tc.tile_pool(name="small", bufs=4) as spool,
            tc.tile_pool(name="bigps", bufs=5, space="PSUM") as bigp,
            tc.tile_pool(name="smallps", bufs=3, space="PSUM") as smp,
        ):
            # ---- constants (no DMA needed) ---------------------------------
            onesc = pers.tile([128, 128], BF16)
            nc.vector.memset(onesc[:], 1.0)
            ones2 = pers.tile([128, 2, 1], FP8)
            nc.vector.memset(ones2[:], 1.0)
            eps8 = pers.tile([GPT, 1], F32)
            nc.vector.memset(eps8[:], EPS)
            # Pre-trigger the scalar engine's Sqrt activation-table load
            # (1.28us) while the PE warm-up runs; without this it fires on
            # the first real Sqrt, mid GroupNorm chain.
            scr8 = pers.tile([GPT, 1], F32)
            nc.scalar.activation(
                scr8[:], eps8[:], mybir.ActivationFunctionType.Sqrt
            )

            warm_ps = bigp.tile([128, 512], F32, tag="ps")

            # HAM keep-alive: the PE clock gate halves the clock after ~1us
            # of PE idle; cheap 64-wide bf16 matmuls (~53ns) placed at the
            # exact idle windows of the in-order PE queue keep it open.
            def warm(n, after=None):
                for i in range(n):
                    w = nc.tensor.matmul(
                        warm_ps[:, 0:64], onesc[:], onesc[:, 0:64],
                        start=True, stop=True,
                    )
                    if after is not None and i == 0:
                        add_dep_helper(w.ins, after.ins, sync=True,
                                       reason="pin filler batch after anchor")

            # ---- x (bf16), one DMA per channel tile ------------------------
            x_sb = pers.tile([128, CT, NPIX], FP8)
            x_dmas = []
            for ct in range(CT):
                x_dmas.append(nc.sync.dma_start(x_sb[:, ct, :], x_d[:, ct, :]))

            # ---- tiny loads ------------------------------------------------
            gnw_sb = pers.tile([128, CT], F32)
            nc.sync.dma_start(gnw_sb[:], gnw_d[:])
            gnb_sb = pers.tile([128, CT], F32)
            nc.sync.dma_start(gnb_sb[:], gnb_d[:])
            ind1_sb = pers.tile([128, GPT], F32)
            nc.sync.dma_start(ind1_sb[:], ind1_d[:])
            ind2_sb = pers.tile([GPT, 128], F32)
            nc.sync.dma_start(ind2_sb[:], ind2_d[:])

            # ---- weights (fp8: 256KB each), serialized behind x ------------
            qa_sb = pers.tile([128, CT, C], FP8)
            d = nc.sync.dma_start(qa_sb[:], qa_d[:])
            add_dep_helper(d.ins, x_dmas[-1].ins, sync=True,
                           reason="x first on the DMA rings")
            vw_sb = pers.tile([128, CT, C], FP8)
            dvw = nc.sync.dma_start(vw_sb[:], vw_d[:])
            add_dep_helper(dvw.ins, x_dmas[-1].ins, sync=True,
                           reason="x first on the DMA rings")
            if with_qbias:
                rw_sb = pers.tile([128, CT, 1], FP8)
                d = nc.sync.dma_start(rw_sb[:], rw_d[:])
                add_dep_helper(d.ins, x_dmas[-1].ins, sync=True,
                               reason="x first on the DMA rings")

            # ---- residual (+proj bias), transposed; needed only at the end -
            xpb_sb = pers.tile([128, JT, C], BF16)
            for half in range(4):
                d = nc.sync.dma_start(
                    xpb_sb[:, 2 * half : 2 * half + 2, :],
                    xpb_d[:, 2 * half : 2 * half + 2, :],
                )
                add_dep_helper(d.ins, dvw.ins, sync=True,
                               reason="weights first on the DMA rings")

            warm(WARM0)

            # ---- group norm, fully pipelined per channel tile --------------
            xn_sb = pers.tile([128, CT, NPIX], FP8)
            chA = pers.tile([128, CT], F32)
            chB = pers.tile([128, CT], F32)
            for ct in range(CT):
                st6 = spool.tile([128, 2, 6], F32, tag="st6")
                nc.vector.bn_stats(st6[:, 0, :], x_sb[:, ct, 0:512])
                nc.vector.bn_stats(st6[:, 1, :], x_sb[:, ct, 512:1024])
                mv = spool.tile([128, 2], F32, tag="mv")
                nc.vector.bn_aggr(mv[:], st6[:])
                # statc = [mean, E[x^2]] per channel (sbuf->sbuf: gpsimd)
                statc = spool.tile([128, 2], F32, tag="statc")
                nc.gpsimd.tensor_copy(statc[:, 0:1], mv[:, 0:1])
                nc.gpsimd.tensor_mul(statc[:, 1:2], mv[:, 0:1], mv[:, 0:1])
                nc.gpsimd.tensor_add(statc[:, 1:2], statc[:, 1:2], mv[:, 1:2])
                # group-combine for this tile's 8 groups
                gsp = smp.tile([GPT, 2], F32, tag="sps")
                gsum_mm = nc.tensor.matmul(
                    gsp[:], ind1_sb[:], statc[:], start=True, stop=True
                )
                gs = spool.tile([GPT, 2], F32, tag="gs")
                nc.scalar.activation(
                    gs[:], gsp[:], mybir.ActivationFunctionType.Identity
                )
                gvar = spool.tile([GPT, 1], F32, tag="gvar")
                nc.gpsimd.tensor_mul(gvar[:], gs[:, 0:1], gs[:, 0:1])
                nc.gpsimd.tensor_sub(gvar[:], gs[:, 1:2], gvar[:])
                grow = spool.tile([GPT, 2], F32, tag="grow")
                gstd = spool.tile([GPT, 1], F32, tag="gstd")
                nc.scalar.activation(
                    gstd[:], gvar[:], mybir.ActivationFunctionType.Sqrt,
                    bias=eps8[:],
                )
                nc.vector.reciprocal(grow[:, 0:1], gstd[:])
                nc.vector.scalar_tensor_tensor(
                    out=grow[:, 1:2], in0=gs[:, 0:1], scalar=-1.0,
                    in1=grow[:, 0:1],
                    op0=mybir.AluOpType.mult, op1=mybir.AluOpType.mult,
                )
                warm(WARMI)  # fill the PE while the chain above runs
                # broadcast to channels; fold gn weight/bias:  xn = x*A + B
                bcp = bigp.tile([128, 2], F32, tag="ps")
                nc.tensor.matmul(bcp[:], ind2_sb[:], grow[:], start=True, stop=True)
                nc.vector.tensor_scalar(
                    out=chA[:, ct : ct + 1], in0=bcp[:, 0:1],
                    scalar1=gnw_sb[:, ct : ct + 1], scalar2=None,
                    op0=mybir.AluOpType.mult,
                )
                nc.vector.scalar_tensor_tensor(
                    out=chB[:, ct : ct + 1], in0=bcp[:, 1:2],
                    scalar=gnw_sb[:, ct : ct + 1], in1=gnb_sb[:, ct : ct + 1],
                    op0=mybir.AluOpType.mult, op1=mybir.AluOpType.add,
                )
                # normalize early tiles on gpsimd inline; the late tiles go
                # to the scalar engine AFTER the loop so their heavy writes
                # never sit ahead of a later tile's Sqrt in the queue
                if ct < 2:
                    nc.gpsimd.tensor_scalar(
                        out=xn_sb[:, ct, :],
                        in0=x_sb[:, ct, :],
                        scalar1=chA[:, ct : ct + 1],
                        scalar2=chB[:, ct : ct + 1],
                        op0=mybir.AluOpType.mult,
                        op1=mybir.AluOpType.add,
                    )
                if ct == 2:
                    anchor = gsum_mm

            for ct in (2, 3):
                nc.scalar.activation(
                    xn_sb[:, ct, :], x_sb[:, ct, :],
                    mybir.ActivationFunctionType.Identity,
                    scale=chA[:, ct : ct + 1], bias=chB[:, ct : ct + 1],
                )

            warm(WARMT, after=anchor)  # fill the PE while the gn tail runs

            # ---- t = M^T xn  (fp8 DoubleRow), nh-major for early S ---------
            t_sb = pers.tile([128, CT, NPIX], FP8)
            for nh in range(NH):
                for co in range(CT):
                    ps = bigp.tile([128, 512], F32, tag="ps")
                    for k in range(2):
                        nc.tensor.matmul(
                            ps[:],
                            qa_sb[:, 2 * k : 2 * k + 2, co * 128 : (co + 1) * 128],
                            xn_sb[:, 2 * k : 2 * k + 2, nh * 512 : (nh + 1) * 512],
                            start=(k == 0), stop=(k == 1), perf_mode=DR,
                        )
                    nc.vector.tensor_copy(
                        t_sb[:, co, nh * 512 : (nh + 1) * 512], ps[:]
                    )

            # ---- r[j] = scale * bq . k_j  (only when q-bias nonzero) -------
            if with_qbias:
                r_sb = pers.tile([128, JT], F32)
                for jt in range(JT):
                    rp = smp.tile([128, 1], F32, tag="sps")
                    for k in range(2):
                        nc.tensor.matmul(
                            rp[:],
                            xn_sb[:, 2 * k : 2 * k + 2, jt * 128 : (jt + 1) * 128],
                            rw_sb[:, 2 * k : 2 * k + 2, :],
                            start=(k == 0), stop=(k == 1), perf_mode=DR,
                        )
                    nc.vector.tensor_copy(r_sb[:, jt : jt + 1], rp[:])

            # ---- E[j, i] = exp(scale * S[i, j]); v~ between the nh halves --
            # PE order: S(nh0) -> v~ -> S(nh1) -> att; the v~ matmuls keep the
            # PE busy while the scalar engine works through the nh0 exps.
            e_sb = pers.tile([128, JT, NPIX], FP8)
            vt_sb = pers.tile([128, JT, C], FP8)

            def s_half(nh):
                for jt in range(JT):
                    ps = bigp.tile([128, 512], F32, tag="ps")
                    for k in range(2):
                        nc.tensor.matmul(
                            ps[:],
                            xn_sb[:, 2 * k : 2 * k + 2, jt * 128 : (jt + 1) * 128],
                            t_sb[:, 2 * k : 2 * k + 2, nh * 512 : (nh + 1) * 512],
                            start=(k == 0), stop=(k == 1), perf_mode=DR,
                        )
                    bias = r_sb[:, jt : jt + 1] if with_qbias else 0.0
                    nc.scalar.activation(
                        e_sb[:, jt, nh * 512 : (nh + 1) * 512], ps[:],
                        mybir.ActivationFunctionType.Exp,
                        scale=SCALE, bias=bias,
                    )

            s_half(0)
            for jt in range(JT):
                ps = bigp.tile([128, 512], F32, tag="ps")
                for k in range(2):
                    nc.tensor.matmul(
                        ps[:],
                        xn_sb[:, 2 * k : 2 * k + 2, jt * 128 : (jt + 1) * 128],
                        vw_sb[:, 2 * k : 2 * k + 2, :],
                        start=(k == 0), stop=(k == 1), perf_mode=DR,
                    )
                nc.vector.tensor_copy(vt_sb[:, jt, :], ps[:])
            s_half(1)

            # ---- att^T[i, c] = sum_j E[j, i] v~^T[j, c]; denominators as
            # [128,1] psums; evac = (ps * 1/D) + (x^T + pb), streamed out ----
            rc_sb = pers.tile([128, JT], F32)
            y_sb = pers.tile([128, JT, C], BF16)
            for jt in range(JT):
                dps = smp.tile([128, 1], F32, tag="sps")
                for k in range(4):
                    nc.tensor.matmul(
                        dps[:],
                        e_sb[:, 2 * k : 2 * k + 2, jt * 128 : (jt + 1) * 128],
                        ones2[:],
                        start=(k == 0), stop=(k == 3), perf_mode=DR,
                    )
                nc.vector.reciprocal(rc_sb[:, jt : jt + 1], dps[:])
                ps = bigp.tile([128, 512], F32, tag="ps")
                for k in range(4):
                    nc.tensor.matmul(
                        ps[:],
                        e_sb[:, 2 * k : 2 * k + 2, jt * 128 : (jt + 1) * 128],
                        vt_sb[:, 2 * k : 2 * k + 2, :],
                        start=(k == 0), stop=(k == 3), perf_mode=DR,
                    )
                nc.vector.scalar_tensor_tensor(
                    out=y_sb[:, jt, :], in0=ps[:],
                    scalar=rc_sb[:, jt : jt + 1], in1=xpb_sb[:, jt, :],
                    op0=mybir.AluOpType.mult, op1=mybir.AluOpType.add,
                )
                nc.sync.dma_start(y_d[:, jt, :], y_sb[:, jt, :])

    nc.compile()
    return nc


def kernel(x, gn_weight, gn_bias, qkv_w, qkv_b, proj_w, proj_b):
    global LAST_RESULTS
    b, c, h, w = x.shape
    assert (b, c, h * w) == (8, C, NPIX)

    f8np = mybir.dt.np(FP8)
    bf16np = mybir.dt.np(BF16)
    x = np.asarray(x, np.float32)
    qkv_b = np.asarray(qkv_b, np.float32)
    qkv_w = np.asarray(qkv_w, np.float32)
    proj_w = np.asarray(proj_w, np.float32)
    # A nonzero q-bias contributes a per-key softmax term r[j] = bq.k_j;
    # k-bias and v-bias fold away (softmax shift invariance / rows sum to 1).
    with_qbias = bool(np.any(qkv_b[0:C]))

    if ("nc", with_qbias) not in _cache:
        _cache[("nc", with_qbias)] = _build(with_qbias)
    nc = _cache[("nc", with_qbias)]

    def col(v):  # [512] vector -> [128, CT] per-partition columns
        return np.ascontiguousarray(np.asarray(v, np.float32).reshape(CT, 128).T)

    def wtile(wT):  # [c_in, cols] -> [128, CT, cols] fp8
        return np.ascontiguousarray(
            np.asarray(wT).reshape(CT, 128, -1).transpose(1, 0, 2).astype(f8np)
        )

    Wq, Wk, Wv = qkv_w[0:C], qkv_w[C : 2 * C], qkv_w[2 * C :]
    M = Wq.astype(np.float64).T @ Wk.astype(np.float64)          # [c_in, c_out]
    WtT = (proj_w.astype(np.float64) @ Wv.astype(np.float64)).T  # [c_in, c_out]
    pb_eff = proj_b + proj_w @ qkv_b[2 * C :]

    shared = {
        "qa": wtile(M),
        "vw": wtile(WtT),
        "gnw": col(gn_weight),
        "gnb": col(gn_bias),
    }
    if with_qbias:
        rw = SCALE * (Wk.astype(np.float64).T @ qkv_b[0:C].astype(np.float64))
        shared["rw"] = np.ascontiguousarray(
            rw.reshape(CT, 128, 1).transpose(1, 0, 2).astype(f8np)
        )

    xs = x.reshape(b, CT, 128, NPIX)
    xt = x.reshape(b, C, NPIX).transpose(0, 2, 1)  # [b, pix, c]
    in_maps = [
        {
            "x": np.ascontiguousarray(xs[i].transpose(1, 0, 2)).astype(f8np),
            "xpb": np.ascontiguousarray(
                (xt[i] + pb_eff).reshape(JT, 128, C).transpose(1, 0, 2)
            ).astype(bf16np),
            **shared,
        }
        for i in range(b)
    ]

    res = run_bass_kernel_spmd(
        nc, in_maps, core_ids=list(range(8)), trace=TRACE, **TRACE_KW
    )
    LAST_RESULTS = res
    out = np.stack(
        [
            r["y"].astype(np.float32).transpose(1, 0, 2)
            .reshape(NPIX, C).T.reshape(c, h, w)
            for r in res.results
        ]
    )
    return np.ascontiguousarray(out).astype(np.float32)
